# revision 1
# baseline (speedup 1.0000x reference)
# MoE routing + sparse-frequency inverse FFT2 kernel for Trainium2 (8 NeuronCores).
#
# Math: out_b = ALPHA * Re(ifft2(mask_b)) where mask_b has 4096 nonzero
# frequencies (top-2 experts x 2048 each).  With the symmetric real DFT basis
#   C[x,u] = cos(2*pi*x*u/768)/768,  S[x,u] = sin(2*pi*x*u/768)/768
# the dense iFFT2 factorizes into four 768^3 matmuls per sample:
#   out = (C @ (M @ C) - S @ (M @ S)) * ALPHA
# Device work per core (4 samples): router GEMM, top-2 selection and weights,
# per-expert entry gather (row-granular indirect DMA at offsets computed from
# the one-hot expert selection), sparse->dense mask build via iota/compare
# one-hots placed with PE matmuls, then the four big matmuls in float32r.
# Host only prepares input-layout constants: the C/S tables and a bucketed,
# padded, partition-major re-layout of the static (list_indices, coeff)
# tables, plus batch sharding.
#
# Element-granular DMA scatter is avoided on purpose: TRN2's indirect DMA is
# row-granular (one offset per partition, contiguous run per partition), so
# the mask is built from gathered (u, v, val) entry groups instead.

import sys

sys.path.insert(0, "/opt/trn_rl_repo")

import numpy as np

import concourse.bacc as bacc
import concourse.mybir as mybir
import concourse.tile as tile
from concourse.bass import IndirectOffsetOnAxis
from concourse.bass_utils import run_bass_kernel_spmd
from concourse.masks import make_identity

N = 768
E = 64
NF = 2048
B = 32
NCORES = 8
BPC = B // NCORES          # samples per core
NBLK = 6                   # 768 / 128
ALPHA = 300.0
GRID = N * N
HALF = N // 2 + 2          # 386 computed stage-1 columns (even width for f32r)

# per-(expert, v-chunk) buckets, sub-bucketed by u-range so each build matmul
# is one bank-aligned instruction: u in [0,512) padded to 384, u in [512,768)
# padded to 256.  Pads sit >=11 sigma above the expected bucket fills.
HB = ((0, 512, 384), (512, 256, 256))   # (u0, uwidth, pad)
BROW = sum(p for _, _, p in HB)          # 640 entries per (expert, v-chunk)
EROW = NBLK * BROW                       # 3840 entries per expert
COLS = EROW // 128                       # 30 gather columns per expert slot

F32 = mybir.dt.float32
F32R = mybir.dt.float32r
I32 = mybir.dt.int32
AOT = mybir.AluOpType

KERNEL_TRACE = False       # test harness can flip this to profile
LAST_RESULT = None

_NC = None


def _build():
    nc = bacc.Bacc(trn_type="TRN2")

    cls4 = nc.dram_tensor("cls4", [BPC, N], F32, kind="ExternalInput")
    wr = nc.dram_tensor("wr", [E, N], F32, kind="ExternalInput")
    br = nc.dram_tensor("br", [E], F32, kind="ExternalInput")
    u2 = nc.dram_tensor("u2", [E, EROW], F32, kind="ExternalInput")
    vm2 = nc.dram_tensor("vm2", [E, EROW], F32, kind="ExternalInput")
    cv2 = nc.dram_tensor("cv2", [E, EROW], F32, kind="ExternalInput")
    bases = nc.dram_tensor("bases", [E, 1], F32, kind="ExternalInput")
    jm = nc.dram_tensor("jm", [128, 128], F32R, kind="ExternalInput")
    ct = nc.dram_tensor("ct", [N, N], F32R, kind="ExternalInput")
    st = nc.dram_tensor("st", [N, N], F32R, kind="ExternalInput")
    out4 = nc.dram_tensor("out4", [BPC, N, N], F32, kind="ExternalOutput")

    with tile.TileContext(nc) as tc:
        with (
            tc.tile_pool(name="const", bufs=1) as cpool,
            tc.tile_pool(name="tables", bufs=1) as tpool,
            tc.tile_pool(name="routing", bufs=1) as rpool,
            tc.tile_pool(name="gath", bufs=1) as gpool,
            tc.tile_pool(name="build", bufs=20) as bpool,
            tc.tile_pool(name="mt", bufs=2) as mtpool,
            tc.tile_pool(name="pq", bufs=1) as pqpool,
            tc.tile_pool(name="outp", bufs=3) as opool,
            tc.tile_pool(name="psA", bufs=2, space="PSUM") as psA,
            tc.tile_pool(name="psA1", bufs=3, space="PSUM") as psA1,
            tc.tile_pool(name="psB", bufs=1, space="PSUM") as psB,
            tc.tile_pool(name="mir", bufs=2) as mirpool,
            tc.tile_pool(name="t1p", bufs=2) as t1pool,
        ):
            ident = cpool.tile([128, 128], F32)
            make_identity(nc, ident[:])
            ones1 = cpool.tile([1, 128], F32)
            nc.vector.memset(ones1[:], 1.0)
            ones14 = cpool.tile([1, BPC], F32)
            nc.vector.memset(ones14[:], 1.0)
            i768 = cpool.tile([128, N], I32)
            nc.gpsimd.iota(i768[:], pattern=[[1, N]], base=0, channel_multiplier=0)
            i768f = cpool.tile([128, N], F32)
            nc.vector.tensor_copy(i768f[:], i768[:])
            i128f = cpool.tile([128, 128], F32)
            nc.vector.tensor_copy(i128f[:], i768[:, 0:128])
            io24 = cpool.tile([128, 1], I32)
            nc.gpsimd.iota(io24[:], pattern=[[0, 1]], base=0, channel_multiplier=COLS)
            io24f = cpool.tile([128, 1], F32)
            nc.vector.tensor_copy(io24f[:], io24[:])

            br_sb = rpool.tile([1, E], F32)
            nc.sync.dma_start(out=br_sb[:], in_=br[None, :])
            bases_sb = rpool.tile([E, 1], F32)
            nc.sync.dma_start(out=bases_sb[:], in_=bases[:])
            jJ = cpool.tile([128, 128], F32R)
            nc.sync.dma_start(out=jJ[:], in_=jm[:])

            # ---- router: logits = cls4 @ Wr.T + br ----
            comb = rpool.tile([E + BPC, N], F32)
            nc.sync.dma_start(out=comb[0:BPC, :], in_=cls4[:])
            nc.sync.dma_start(out=comb[BPC : BPC + E, :], in_=wr[:])
            combt = rpool.tile([128, NBLK * (E + BPC)], F32)
            for j in range(NBLK):
                tp = psB.tile([128, E + BPC], F32, tag="small")
                nc.tensor.transpose(
                    tp[:],
                    comb[0 : E + BPC, 128 * j : 128 * (j + 1)],
                    ident[0 : E + BPC, 0 : E + BPC],
                )
                nc.scalar.copy(combt[:, (E + BPC) * j : (E + BPC) * (j + 1)], tp[:])
            lg_ps = psB.tile([BPC, E], F32, tag="small")
            for j in range(NBLK):
                base = (E + BPC) * j
                nc.tensor.matmul(
                    lg_ps[:],
                    lhsT=combt[:, base : base + BPC],
                    rhs=combt[:, base + BPC : base + BPC + E],
                    start=(j == 0),
                    stop=False,
                )
            nc.tensor.matmul(
                lg_ps[:], lhsT=ones14[:], rhs=br_sb[:], start=False, stop=True
            )
            logits = rpool.tile([BPC, E], F32)
            nc.vector.tensor_copy(logits[:], lg_ps[:])

            # ---- top-2, renormalized weights, one-hot selectors ----
            max8 = rpool.tile([BPC, 8], F32)
            nc.vector.max(out=max8[:], in_=logits[:])
            l0 = max8[:, 0:1]
            l1 = max8[:, 1:2]
            d = rpool.tile([BPC, 1], F32)
            nc.vector.tensor_sub(d[:], l1, l0)  # l1 - l0
            dT_ps = psB.tile([1, BPC], F32, tag="small")
            nc.tensor.transpose(dT_ps[:], d[:], ident[0:BPC, 0:BPC])
            dT = rpool.tile([1, BPC], F32)
            nc.vector.tensor_copy(dT[:], dT_ps[:])
            w1T = rpool.tile([1, BPC], F32)
            nc.scalar.activation(w1T[:], dT[:], mybir.ActivationFunctionType.Sigmoid)
            w0T = rpool.tile([1, BPC], F32)
            nc.scalar.activation(
                w0T[:], dT[:], mybir.ActivationFunctionType.Sigmoid, scale=-1.0
            )
            oh1 = rpool.tile([BPC, E], F32)
            oh2 = rpool.tile([BPC, E], F32)
            nc.vector.tensor_scalar(oh1[:], logits[:], l0, None, op0=AOT.is_equal)
            nc.vector.tensor_scalar(oh2[:], logits[:], l1, None, op0=AOT.is_equal)
            selT = []
            for srcap in (oh1, oh2):
                sp = psB.tile([E, BPC], F32, tag="small")
                nc.tensor.transpose(sp[:], srcap[:], ident[0:BPC, 0:BPC])
                sbt = rpool.tile([E, BPC], F32, tag=f"selT{len(selT)}")
                nc.vector.tensor_copy(sbt[:], sp[:])
                selT.append(sbt)
            o1T, o2T = selT

            # per-sample scalar rows [1, BPC]: expert table offsets
            eT = []
            for oT in (o1T, o2T):
                ep = psB.tile([1, BPC], F32, tag="small")
                nc.tensor.matmul(
                    ep[:], lhsT=bases_sb[:], rhs=oT[:], start=True, stop=True
                )
                es = rpool.tile([1, BPC], F32, tag=f"eT{len(eT)}")
                nc.vector.tensor_copy(es[:], ep[:])
                eT.append(es)

            # broadcast each scalar row to all 128 partitions: [128, BPC]
            bc = []
            for rowap in (eT[0], eT[1], w0T, w1T):
                bp = psB.tile([128, BPC], F32, tag="small")
                nc.tensor.matmul(
                    bp[:], lhsT=ones1[:], rhs=rowap[:], start=True, stop=True
                )
                bs = rpool.tile([128, BPC], F32, tag=f"bc{len(bc)}")
                nc.vector.tensor_copy(bs[:], bp[:])
                bc.append(bs)
            ebc = bc[0:2]    # expert base offsets per slot
            wbc = bc[2:4]    # expert weights per slot

            # ---- C/S table loads AFTER the routing-phase emission so the
            # small router DMAs aren't queued behind 4.7 MB on the sync FIFO
            ct_sb = tpool.tile([128, NBLK * N], F32R, tag="ct")
            st_sb = tpool.tile([128, NBLK * N], F32R, tag="st")
            for j in range(NBLK):
                nc.sync.dma_start(
                    out=ct_sb[:, N * j : N * (j + 1)],
                    in_=ct[128 * j : 128 * (j + 1), :],
                )
                nc.sync.dma_start(
                    out=st_sb[:, N * j : N * (j + 1)],
                    in_=st[128 * j : 128 * (j + 1), :],
                )

            ych = [(0, 512), (512, 256)]

            def emit_d(b, di, mc):
                dps = psA.tile([128, N], F32, tag="mm")
                for c0, cw in ych:
                    nc.tensor.matmul(
                        dps[:, c0 : c0 + cw],
                        lhsT=jJ[:],
                        rhs=mc[:, c0 : c0 + cw],
                        start=True, stop=True,
                    )
                ob = opool.tile([128, N], F32, tag="ob")
                nc.scalar.copy(ob[:], dps[:])
                nc.scalar.dma_start(
                    out=out4[:][b][128 * (4 + di) : 128 * (5 + di), :], in_=ob[:]
                )

            # ---- gather ALL samples' (u, vm, coeff) entry tables upfront ----
            allg = []
            for b in range(BPC):
                gus, gvms, gcws = [], [], []
                for slot in range(2):
                    offf = gpool.tile([128, 1], F32, tag="offf")
                    nc.vector.tensor_add(offf[:], ebc[slot][:, b : b + 1], io24f[:])
                    offs = gpool.tile([128, 1], I32, tag="offs")
                    nc.vector.tensor_copy(offs[:], offf[:])
                    gu = gpool.tile([128, COLS], F32, tag=f"gu{b}_{slot}")
                    gvm = gpool.tile([128, COLS], F32, tag=f"gvm{b}_{slot}")
                    gcv = gpool.tile([128, COLS], F32, tag=f"gcv{b}_{slot}")
                    for tab, dst in ((u2, gu), (vm2, gvm), (cv2, gcv)):
                        nc.gpsimd.indirect_dma_start(
                            out=dst[:],
                            out_offset=None,
                            in_=tab[:],
                            in_offset=IndirectOffsetOnAxis(ap=offs[:], axis=1),
                        )
                    gcw = gpool.tile([128, COLS], F32, tag=f"gcw{b}_{slot}")
                    nc.vector.tensor_scalar(
                        gcw[:], gcv[:], wbc[slot][:, b : b + 1], None, op0=AOT.mult
                    )
                    gus.append(gu)
                    gvms.append(gvm)
                    gcws.append(gcw)
                allg.append((gus, gvms, gcws))

            for b in range(BPC):
                gus, gvms, gcws = allg[b]
                # ---- build MT (transposed mask) chunk by chunk on PE ----
                mt_sb = mtpool.tile([128, NBLK * N], F32R, tag="mt")
                for j in range(NBLK):
                    mtps = psA.tile([128, N], F32, tag="mm")
                    colbase = COLS * j // NBLK * 0  # columns laid out per j below
                    for hi, (u0, uw, pad) in enumerate(HB):
                        ng = pad // 128
                        coff = 5 * j + (0 if hi == 0 else HB[0][2] // 128)
                        for slot in range(2):
                            for g in range(ng):
                                col = coff + g
                                voh = bpool.tile([128, 128], F32R, tag="voh")
                                nc.vector.tensor_scalar(
                                    voh[:], i128f[:], gvms[slot][:, col : col + 1],
                                    None, op0=AOT.is_equal,
                                )
                                rhsb = bpool.tile([128, 512], F32R, tag="rhsb")
                                nc.vector.tensor_scalar(
                                    rhsb[:, 0:uw], i768f[:, u0 : u0 + uw],
                                    gus[slot][:, col : col + 1],
                                    gcws[slot][:, col : col + 1],
                                    op0=AOT.is_equal, op1=AOT.mult,
                                )
                                nc.tensor.matmul(
                                    mtps[:, u0 : u0 + uw],
                                    lhsT=voh[:],
                                    rhs=rhsb[:, 0:uw],
                                    start=(slot == 0 and g == 0),
                                    stop=(slot == 1 and g == ng - 1),
                                )
                    nc.scalar.copy(mt_sb[:, N * j : N * (j + 1)], mtps[:])

                # ---- stage 1 (paired): P = 300*(M @ C), Qn = -300*(M @ S) ----
                # only columns [0, HALF) are computed; C-column symmetry gives
                # P[:, N-y] = P[:, y] and Qn[:, N-y] = -Qn[:, y].
                p_sb = pqpool.tile([128, NBLK * N], F32R, tag="p")
                q_sb = pqpool.tile([128, NBLK * N], F32R, tag="q")
                for i in range(NBLK):
                    pps = psA1.tile([128, HALF], F32, tag="mm1")
                    qps = psA1.tile([128, HALF], F32, tag="mm1")
                    for k in range(NBLK):
                        lhs = mt_sb[:, N * k + 128 * i : N * k + 128 * (i + 1)]
                        nc.tensor.matmul(
                            pps[:], lhsT=lhs, rhs=ct_sb[:, N * k : N * k + HALF],
                            start=(k == 0), stop=(k == NBLK - 1),
                        )
                        nc.tensor.matmul(
                            qps[:], lhsT=lhs, rhs=st_sb[:, N * k : N * k + HALF],
                            start=(k == 0), stop=(k == NBLK - 1),
                        )
                    nc.scalar.mul(p_sb[:, N * i : N * i + HALF], pps[:], ALPHA)
                    nc.scalar.mul(q_sb[:, N * i : N * i + HALF], qps[:], -ALPHA)
                    nc.scalar.copy(
                        p_sb[:, N * i + HALF : N * (i + 1)],
                        p_sb[:][:, N * i + (N - HALF) : N * i : -1],
                    )
                    nc.scalar.mul(
                        q_sb[:, N * i + HALF : N * (i + 1)],
                        q_sb[:][:, N * i + (N - HALF) : N * i : -1],
                        -1.0,
                    )

                # ---- stage 2: rows 0..511 as T1+T2; rows 512..767 mirrored ----
                # T1 = C @ P, T2 = S @ Qn (both already x300).  Row symmetry:
                # out[N-x] = T1[x] - T2[x], realized with shifted anti-identity
                # matmuls (jA, jB) on M_i = T1_i - T2_i.
                mirs = []
                for i in range(4):
                    t1 = psA.tile([128, N], F32, tag="mm")
                    t2 = psA.tile([128, N], F32, tag="mm")
                    for dst, tbl, srcm in ((t1, ct_sb, p_sb), (t2, st_sb, q_sb)):
                        for k in range(NBLK):
                            for c0, cw in ych:
                                nc.tensor.matmul(
                                    dst[:, c0 : c0 + cw],
                                    lhsT=tbl[:, N * k + 128 * i : N * k + 128 * (i + 1)],
                                    rhs=srcm[:, N * k + c0 : N * k + c0 + cw],
                                    start=(k == 0),
                                    stop=(k == NBLK - 1),
                                )
                    t1s = t1pool.tile([128, N], F32, tag="t1")
                    nc.scalar.copy(t1s[:], t1[:])
                    ob = opool.tile([128, N], F32, tag="ob")
                    nc.vector.tensor_tensor(ob[:], t1s[:], t2[:], op=AOT.add)
                    nc.scalar.dma_start(
                        out=out4[:][b][128 * i : 128 * (i + 1), :], in_=ob[:]
                    )
                    # mirror source tiles: mc[d] rows = T1-T2 at x = (2-d)*128 - m
                    if i == 0:
                        m = mirpool.tile([128, N], F32R, tag="mc1")
                        nc.vector.tensor_tensor(m[:], t1s[:], t2[:], op=AOT.subtract)
                        mirs.append(m)  # mc2 body (block 0), row 0 patched later
                    elif i == 1:
                        m = mirpool.tile([128, N], F32R, tag="mc0")
                        nc.vector.tensor_tensor(m[:], t1s[:], t2[:], op=AOT.subtract)
                        mirs.append(m)  # mc1 body (block 1), row 0 patched later
                        nc.vector.tensor_tensor(
                            mirs[0][0:1, :], t1s[0:1, :], t2[0:1, :], op=AOT.subtract
                        )  # mc2 row 0 = block-1 row 0 (x = 128)
                    elif i == 2:
                        nc.vector.tensor_tensor(
                            mirs[1][0:1, :], t1s[0:1, :], t2[0:1, :], op=AOT.subtract
                        )  # mc1 row 0 = block-2 row 0 (x = 256)
                emit_d(b, 0, mirs[1])
                emit_d(b, 1, mirs[0])

    nc.compile()
    return nc


def _get_nc():
    global _NC
    if _NC is None:
        _NC = _build()
    return _NC


def _host_tables():
    a = np.arange(N, dtype=np.int64)
    ang = (2.0 * np.pi / N) * ((a[:, None] * a[None, :]) % N)
    ctv = (np.cos(ang) / N).astype(np.float32)
    stv = (np.sin(ang) / N).astype(np.float32)
    return ctv, stv


def _host_entry_tables(list_indices, coeff):
    """Bucket each expert's (u, v, coeff) entries by v-chunk, pad buckets to
    PAD, and lay out partition-major (entry 128*g + p lands at column g of
    partition p's contiguous gather run)."""
    li = list_indices.astype(np.int64)
    uu = li // N
    vv = li % N
    u2 = np.zeros((E, EROW), np.float32)
    vm2 = np.full((E, EROW), -9.0, np.float32)
    cv2 = np.zeros((E, EROW), np.float32)
    for e in range(E):
        for j in range(NBLK):
            selj = vv[e] // 128 == j
            base = BROW * j
            for u0, uw, pad in HB:
                sel = np.where(selj & (uu[e] >= u0) & (uu[e] < u0 + uw))[0]
                cnt = len(sel)
                assert cnt <= pad, f"bucket overflow: e{e} j{j} u{u0}: {cnt}"
                u2[e, base : base + cnt] = uu[e, sel]
                vm2[e, base : base + cnt] = vv[e, sel] - 128 * j
                cv2[e, base : base + cnt] = coeff[e, sel]
                base += pad
    # partition-major runs: table[e, p*COLS + g] = arr[e, 128*g + p]
    perm = np.array([128 * g + p for p in range(128) for g in range(COLS)])
    return u2[:, perm], vm2[:, perm], cv2[:, perm]


def kernel(cls_token, W_router, b_router, coeff, list_indices):
    global LAST_RESULT
    cls_token = np.asarray(cls_token)
    W_router = np.asarray(W_router)
    b_router = np.asarray(b_router)
    coeff = np.asarray(coeff)
    list_indices = np.asarray(list_indices)
    assert cls_token.shape == (B, N) and coeff.shape == (E, NF)
    nc = _get_nc()
    ctv, stv = _host_tables()
    u2v, vm2v, cv2v = _host_entry_tables(list_indices, coeff)
    basesv = (np.arange(E, dtype=np.float32) * EROW).reshape(E, 1)
    jmv = np.zeros((128, 128), np.float32)
    for m_ in range(128):
        jmv[(128 - m_) % 128, m_] = 1.0
    wrr = np.ascontiguousarray(W_router, dtype=np.float32)
    brr = np.ascontiguousarray(b_router, dtype=np.float32)
    in_maps = []
    for c in range(NCORES):
        in_maps.append(
            {
                "cls4": np.ascontiguousarray(
                    cls_token[BPC * c : BPC * (c + 1)], dtype=np.float32
                ),
                "wr": wrr,
                "br": brr,
                "u2": u2v,
                "vm2": vm2v,
                "cv2": cv2v,
                "bases": basesv,
                "jm": jmv,
                "ct": ctv,
                "st": stv,
            }
        )
    res = run_bass_kernel_spmd(
        nc, in_maps, core_ids=list(range(NCORES)), trace=KERNEL_TRACE
    )
    LAST_RESULT = res
    out = np.concatenate([res.results[c]["out4"] for c in range(NCORES)], axis=0)
    return out



# revision 5
# speedup vs baseline: 1.3369x; 1.3369x over previous
# MoE routing + sparse-frequency inverse FFT2 kernel for Trainium2 (8 NeuronCores).
#
# Math: out_b = ALPHA * Re(ifft2(mask_b)) where mask_b has 4096 nonzero
# frequencies (top-2 experts x 2048 each).  With the symmetric real DFT basis
#   C[x,u] = cos(2*pi*x*u/768)/768,  S[x,u] = sin(2*pi*x*u/768)/768
# the dense iFFT2 factorizes as out = C @ (M @ C) - S @ (M @ S), all x300.
# This version folds the u-symmetry of the basis into the contraction:
#   sum_u C[x,u] P[u,y] = sum_{u=0..383} C[x,u] (P[u]+P[768-u]) + C[x,384] P[384]
# (S odd: same with a minus fold and no u=384 term), halving both the stage-1
# output rows and the stage-2 contraction depth.  All heavy matmuls run in
# bf16 (exact for the one-hot/index operands; ~4e-3 rel err overall).
#
# Device work per core (4 samples): router GEMM + top-2 (fp32), per-expert
# entry gather (indirect DMA from bf16 tables bucketed by (v-chunk, u-chunk),
# 128 entries per bucket), mask-transpose build via one-hot PE matmuls,
# u-fold on DVE (reversed-stride adds into a zero-padded column layout),
# stage-1/stage-2 bf16 matmuls with the u=384 orphan handled as a rank-1
# PSUM-accumulated tile.  Output rows are computed directly (no x-mirror).

import sys

sys.path.insert(0, "/opt/trn_rl_repo")

import numpy as np
import ml_dtypes

import concourse.bacc as bacc
import concourse.mybir as mybir
import concourse.tile as tile
from concourse.bass import IndirectOffsetOnAxis
from concourse.bass_utils import run_bass_kernel_spmd
from concourse.masks import make_identity

N = 768
E = 64
NF = 2048
B = 32
NCORES = 8
BPC = B // NCORES          # samples per core
NBLK = 6                   # 768 / 128
ALPHA = 300.0
HALF = N // 2 + 2          # 386 computed stage-1 columns
PAD = 128                  # entries per (v-chunk, u-chunk) bucket
NB = NBLK * NBLK           # 36 buckets per expert
EROW = NB * PAD            # 4608 entries per expert slot
COLS = EROW // 128         # 36 gather columns per expert slot
CHW = N + 4                # mask-transpose chunk width (4 zero pad cols)

# packed C/S table layout: chunks 0..2 full width, 3..5 only HALF cols
CT_OFF = [0, N, 2 * N, 3 * N, 3 * N + HALF, 3 * N + 2 * HALF]
CT_W = 3 * N + 3 * HALF    # 3462

F32 = mybir.dt.float32
BF16 = mybir.dt.bfloat16
I32 = mybir.dt.int32
AOT = mybir.AluOpType
ACT = mybir.ActivationFunctionType

KERNEL_TRACE = False       # test harness can flip this to profile
LAST_RESULT = None

_NC = None


def _build():
    nc = bacc.Bacc(trn_type="TRN2")

    cls4 = nc.dram_tensor("cls4", [BPC, N], F32, kind="ExternalInput")
    wr = nc.dram_tensor("wr", [E, N], F32, kind="ExternalInput")
    br = nc.dram_tensor("br", [E], F32, kind="ExternalInput")
    u2 = nc.dram_tensor("u2", [E, EROW], F32, kind="ExternalInput")
    vm2 = nc.dram_tensor("vm2", [E, EROW], F32, kind="ExternalInput")
    cv2 = nc.dram_tensor("cv2", [E, EROW], F32, kind="ExternalInput")
    bases = nc.dram_tensor("bases", [E, 1], F32, kind="ExternalInput")
    ctp = nc.dram_tensor("ctp", [128, CT_W], BF16, kind="ExternalInput")
    stp = nc.dram_tensor("stp", [128, CT_W], BF16, kind="ExternalInput")
    c384 = nc.dram_tensor("c384", [1, N], BF16, kind="ExternalInput")
    out4 = nc.dram_tensor("out4", [BPC, N, N], F32, kind="ExternalOutput")

    with tile.TileContext(nc) as tc:
        with (
            tc.tile_pool(name="const", bufs=1) as cpool,
            tc.tile_pool(name="tables", bufs=1) as tpool,
            tc.tile_pool(name="routing", bufs=1) as rpool,
            tc.tile_pool(name="gath", bufs=1) as gpool,
            tc.tile_pool(name="build", bufs=2) as bpool,
            tc.tile_pool(name="mt", bufs=2) as mtpool,
            tc.tile_pool(name="ms", bufs=2) as mspool,
            tc.tile_pool(name="pq", bufs=2) as pqpool,
            tc.tile_pool(name="outp", bufs=3) as opool,
            tc.tile_pool(name="psBig", bufs=2, space="PSUM") as psBig,
            tc.tile_pool(name="psS1", bufs=3, space="PSUM") as psS1,
        ):
            ident = cpool.tile([128, 128], F32)
            make_identity(nc, ident[:])
            ones1 = cpool.tile([1, 128], F32)
            nc.vector.memset(ones1[:], 1.0)
            ones14 = cpool.tile([1, BPC], F32)
            nc.vector.memset(ones14[:], 1.0)
            i128 = cpool.tile([128, 128], I32)
            nc.gpsimd.iota(i128[:], pattern=[[1, 128]], base=0, channel_multiplier=0)
            i128b = cpool.tile([128, 128], BF16)
            nc.vector.tensor_copy(i128b[:], i128[:])
            io36 = cpool.tile([128, 1], I32)
            nc.gpsimd.iota(io36[:], pattern=[[0, 1]], base=0, channel_multiplier=COLS)
            io36f = cpool.tile([128, 1], F32)
            nc.vector.tensor_copy(io36f[:], io36[:])
            c384pad = cpool.tile([128, N], BF16)
            nc.vector.memset(c384pad[:], 0.0)
            nc.sync.dma_start(out=c384pad[0:1, :], in_=c384[:])
            p384pad = cpool.tile([128, N], BF16)
            nc.vector.memset(p384pad[:], 0.0)

            br_sb = rpool.tile([1, E], F32)
            nc.sync.dma_start(out=br_sb[:], in_=br[None, :])
            bases_sb = rpool.tile([E, 1], F32)
            nc.sync.dma_start(out=bases_sb[:], in_=bases[:])

            # ---- router: logits = cls4 @ Wr.T + br ----
            comb = rpool.tile([E + BPC, N], F32)
            nc.sync.dma_start(out=comb[0:BPC, :], in_=cls4[:])
            nc.sync.dma_start(out=comb[BPC : BPC + E, :], in_=wr[:])
            combt = rpool.tile([128, NBLK * (E + BPC)], F32)
            for j in range(NBLK):
                tp = psS1.tile([128, HALF], F32, tag="s1")
                nc.tensor.transpose(
                    tp[:, 0 : E + BPC],
                    comb[0 : E + BPC, 128 * j : 128 * (j + 1)],
                    ident[0 : E + BPC, 0 : E + BPC],
                )
                nc.scalar.copy(
                    combt[:, (E + BPC) * j : (E + BPC) * (j + 1)], tp[:, 0 : E + BPC]
                )
            lg_ps = psS1.tile([128, HALF], F32, tag="s1")
            for j in range(NBLK):
                base = (E + BPC) * j
                nc.tensor.matmul(
                    lg_ps[0:BPC, 0:E],
                    lhsT=combt[:, base : base + BPC],
                    rhs=combt[:, base + BPC : base + BPC + E],
                    start=(j == 0),
                    stop=False,
                )
            nc.tensor.matmul(
                lg_ps[0:BPC, 0:E], lhsT=ones14[:], rhs=br_sb[:], start=False, stop=True
            )
            logits = rpool.tile([BPC, E], F32)
            nc.vector.tensor_copy(logits[:], lg_ps[0:BPC, 0:E])

            # ---- top-2, renormalized weights, one-hot selectors ----
            max8 = rpool.tile([BPC, 8], F32)
            nc.vector.max(out=max8[:], in_=logits[:])
            l0 = max8[:, 0:1]
            l1 = max8[:, 1:2]
            d = rpool.tile([BPC, 1], F32)
            nc.vector.tensor_sub(d[:], l1, l0)  # l1 - l0
            dT_ps = psS1.tile([128, HALF], F32, tag="s1")
            nc.tensor.transpose(dT_ps[0:1, 0:BPC], d[:], ident[0:BPC, 0:BPC])
            dT = rpool.tile([1, BPC], F32)
            nc.vector.tensor_copy(dT[:], dT_ps[0:1, 0:BPC])
            w1T = rpool.tile([1, BPC], F32)
            nc.scalar.activation(w1T[:], dT[:], ACT.Sigmoid)
            w0T = rpool.tile([1, BPC], F32)
            nc.scalar.activation(w0T[:], dT[:], ACT.Sigmoid, scale=-1.0)
            oh1 = rpool.tile([BPC, E], F32)
            oh2 = rpool.tile([BPC, E], F32)
            nc.vector.tensor_scalar(oh1[:], logits[:], l0, None, op0=AOT.is_equal)
            nc.vector.tensor_scalar(oh2[:], logits[:], l1, None, op0=AOT.is_equal)
            selT = []
            for srcap in (oh1, oh2):
                sp = psS1.tile([128, HALF], F32, tag="s1")
                nc.tensor.transpose(sp[0:E, 0:BPC], srcap[:], ident[0:BPC, 0:BPC])
                sbt = rpool.tile([E, BPC], F32, tag=f"selT{len(selT)}")
                nc.vector.tensor_copy(sbt[:], sp[0:E, 0:BPC])
                selT.append(sbt)
            o1T, o2T = selT

            # per-sample scalar rows [1, BPC]: expert table offsets
            eT = []
            for oT in (o1T, o2T):
                ep = psS1.tile([128, HALF], F32, tag="s1")
                nc.tensor.matmul(
                    ep[0:1, 0:BPC], lhsT=bases_sb[:], rhs=oT[:], start=True, stop=True
                )
                es = rpool.tile([1, BPC], F32, tag=f"eT{len(eT)}")
                nc.vector.tensor_copy(es[:], ep[0:1, 0:BPC])
                eT.append(es)

            # broadcast scalar rows to all 128 partitions: [128, BPC]
            ebc = []
            for rowap in (eT[0], eT[1]):
                bp = psS1.tile([128, HALF], F32, tag="s1")
                nc.tensor.matmul(
                    bp[:, 0:BPC], lhsT=ones1[:], rhs=rowap[:], start=True, stop=True
                )
                bs = rpool.tile([128, BPC], F32, tag=f"ebc{len(ebc)}")
                nc.vector.tensor_copy(bs[:], bp[:, 0:BPC])
                ebc.append(bs)
            wbcb = []
            for rowap in (w0T, w1T):
                bp = psS1.tile([128, HALF], F32, tag="s1")
                nc.tensor.matmul(
                    bp[:, 0:BPC], lhsT=ones1[:], rhs=rowap[:], start=True, stop=True
                )
                bs = rpool.tile([128, BPC], F32, tag=f"wbc{len(wbcb)}")
                nc.vector.tensor_copy(bs[:], bp[:, 0:BPC])
                wbcb.append(bs)

            # ---- C/S table loads AFTER routing-phase emission so the small
            # router DMAs aren't queued behind the big table DMAs
            ct_sb = tpool.tile([128, CT_W], BF16, tag="ct")
            st_sb = tpool.tile([128, CT_W], BF16, tag="st")
            nc.sync.dma_start(out=ct_sb[:], in_=ctp[:])
            nc.sync.dma_start(out=st_sb[:], in_=stp[:])

            # ---- gather ALL samples' (u, vm, coeff) entry tables upfront ----
            allg = []
            for b in range(BPC):
                per_slot = []
                for slot in range(2):
                    offf = gpool.tile([128, 1], F32, tag=f"offf{b}_{slot}")
                    nc.vector.tensor_add(offf[:], ebc[slot][:, b : b + 1], io36f[:])
                    offs = gpool.tile([128, 1], I32, tag=f"offs{b}_{slot}")
                    nc.vector.tensor_copy(offs[:], offf[:])
                    gu = gpool.tile([128, COLS], F32, tag=f"gu{b}_{slot}")
                    gvm = gpool.tile([128, COLS], F32, tag=f"gvm{b}_{slot}")
                    gcv = gpool.tile([128, COLS], F32, tag=f"gcv{b}_{slot}")
                    for tab, dst in ((u2, gu), (vm2, gvm), (cv2, gcv)):
                        nc.gpsimd.indirect_dma_start(
                            out=dst[:],
                            out_offset=None,
                            in_=tab[:],
                            in_offset=IndirectOffsetOnAxis(ap=offs[:], axis=1),
                        )
                    per_slot.append((gu, gvm, gcv))
                allg.append(per_slot)

            for b in range(BPC):
                # ---- one-hot builds for the whole sample, emitted as a block
                # into big double-buffered tiles so DVE runs a sample ahead of
                # the PE instead of stalling on a small buffer ring.
                ohs = []
                for slot in range(2):
                    gu, gvm, gcv = allg[b][slot]
                    voh_all = bpool.tile([128, NB * 128], BF16, tag=f"voh{slot}")
                    rhs_all = bpool.tile([128, NB * 128], BF16, tag=f"rhs{slot}")
                    for bk in range(NB):
                        nc.vector.tensor_scalar(
                            voh_all[:, 128 * bk : 128 * (bk + 1)],
                            i128b[:], gvm[:, bk : bk + 1],
                            wbcb[slot][:, b : b + 1],
                            op0=AOT.is_equal, op1=AOT.mult,
                        )
                        nc.vector.tensor_scalar(
                            rhs_all[:, 128 * bk : 128 * (bk + 1)],
                            i128b[:], gu[:, bk : bk + 1],
                            gcv[:, bk : bk + 1],
                            op0=AOT.is_equal, op1=AOT.mult,
                        )
                    ohs.append((voh_all, rhs_all))

                # ---- build MT (transposed mask) chunk by chunk on PE ----
                # mt chunk j cols [0..767] = M^T[v in chunk j, u]; cols 768..771
                # are zero so the u-fold's reversed read of col 768 sees 0.
                mt_sb = mtpool.tile([128, NBLK * CHW], BF16, tag="mt")
                msym = mspool.tile([128, NBLK * 384], BF16, tag="msym")
                masym = mspool.tile([128, NBLK * 384], BF16, tag="masym")
                for j in range(NBLK):
                    mtps = psBig.tile([128, N], F32, tag="big")
                    for ub in range(NBLK):
                        bk = NBLK * j + ub
                        for slot in range(2):
                            voh_all, rhs_all = ohs[slot]
                            nc.tensor.matmul(
                                mtps[:, 128 * ub : 128 * (ub + 1)],
                                lhsT=voh_all[:, 128 * bk : 128 * (bk + 1)],
                                rhs=rhs_all[:, 128 * bk : 128 * (bk + 1)],
                                start=(slot == 0),
                                stop=(slot == 1),
                            )
                    co = CHW * j
                    nc.scalar.copy(mt_sb[:, co : co + N], mtps[:])
                    nc.vector.memset(mt_sb[:, co + N : co + CHW], 0.0)
                    # ---- u-fold: col c (u=c) += / -= col 768-c (0 for c=0) ----
                    nc.vector.tensor_tensor(
                        msym[:, 384 * j : 384 * (j + 1)],
                        mt_sb[:, co : co + 384],
                        mt_sb[:][:, co + N : co + 384 : -1],
                        op=AOT.add,
                    )
                    nc.vector.tensor_tensor(
                        masym[:, 384 * j : 384 * (j + 1)],
                        mt_sb[:, co : co + 384],
                        mt_sb[:][:, co + N : co + 384 : -1],
                        op=AOT.subtract,
                    )

                # ---- stage 1: P = 300*(Msym @ C), Qn = -300*(Masym @ S) ----
                # Only columns [0, HALF) computed; column symmetry mirrors the
                # rest (P even, Qn odd).  Folded row space is u in [0, 384).
                pq = []
                for mname, msrc, tbl, sgn in (
                    ("pf", msym, ct_sb, 1.0),
                    ("qf", masym, st_sb, -1.0),
                ):
                    xf = pqpool.tile([128, 3 * N], BF16, tag=mname)
                    for i in range(3):
                        pps = psS1.tile([128, HALF], F32, tag="s1")
                        for k in range(NBLK):
                            nc.tensor.matmul(
                                pps[:],
                                lhsT=msrc[:, 384 * k + 128 * i : 384 * k + 128 * (i + 1)],
                                rhs=tbl[:, CT_OFF[k] : CT_OFF[k] + HALF],
                                start=(k == 0),
                                stop=(k == NBLK - 1),
                            )
                        nc.scalar.mul(xf[:, N * i : N * i + HALF], pps[:], sgn * ALPHA)
                        nc.scalar.mul(
                            xf[:, N * i + HALF : N * (i + 1)],
                            xf[:][:, N * i + (N - HALF) : N * i : -1],
                            sgn,
                        )
                    pq.append(xf)
                pf, qf = pq

                # u=384 orphan row (C side only; S row 384 is zero)
                pps384 = psS1.tile([128, HALF], F32, tag="s1")
                for k in range(NBLK):
                    nc.tensor.matmul(
                        pps384[0:1, :],
                        lhsT=mt_sb[:, CHW * k + 384 : CHW * k + 385],
                        rhs=ct_sb[:, CT_OFF[k] : CT_OFF[k] + HALF],
                        start=(k == 0),
                        stop=(k == NBLK - 1),
                    )
                nc.scalar.mul(p384pad[0:1, 0:HALF], pps384[0:1, :], ALPHA)
                nc.scalar.copy(
                    p384pad[0:1, HALF:N], p384pad[0:1, :][:, N - HALF : 0 : -1]
                )

                # ---- stage 2: all 6 row blocks direct, single PSUM accum ----
                for i in range(NBLK):
                    ops = psBig.tile([128, N], F32, tag="big")
                    for c0, cw in ((0, 512), (512, 256)):
                        seq = [
                            (ct_sb[:, N * k + 128 * i : N * k + 128 * (i + 1)],
                             pf[:, N * k + c0 : N * k + c0 + cw])
                            for k in range(3)
                        ]
                        seq.append(
                            (c384pad[:, 128 * i : 128 * (i + 1)],
                             p384pad[:, c0 : c0 + cw])
                        )
                        seq += [
                            (st_sb[:, N * k + 128 * i : N * k + 128 * (i + 1)],
                             qf[:, N * k + c0 : N * k + c0 + cw])
                            for k in range(3)
                        ]
                        for t, (lhsT, rhs) in enumerate(seq):
                            nc.tensor.matmul(
                                ops[:, c0 : c0 + cw],
                                lhsT=lhsT,
                                rhs=rhs,
                                start=(t == 0),
                                stop=(t == len(seq) - 1),
                            )
                    ob = opool.tile([128, N], F32, tag="ob")
                    nc.scalar.copy(ob[:], ops[:])
                    nc.scalar.dma_start(
                        out=out4[:][b][128 * i : 128 * (i + 1), :], in_=ob[:]
                    )

    nc.compile()
    return nc


def _get_nc():
    global _NC
    if _NC is None:
        _NC = _build()
    return _NC


def _host_tables():
    a = np.arange(N, dtype=np.int64)
    ang = (2.0 * np.pi / N) * ((a[:, None] * a[None, :]) % N)
    ctv = (np.cos(ang) / N).astype(np.float32)
    stv = (np.sin(ang) / N).astype(np.float32)
    ctpv = np.zeros((128, CT_W), np.float32)
    stpv = np.zeros((128, CT_W), np.float32)
    for k in range(NBLK):
        w = N if k < 3 else HALF
        ctpv[:, CT_OFF[k] : CT_OFF[k] + w] = ctv[128 * k : 128 * (k + 1), 0:w]
        stpv[:, CT_OFF[k] : CT_OFF[k] + w] = stv[128 * k : 128 * (k + 1), 0:w]
    return ctpv, stpv, ctv[384:385, :]


def _host_entry_tables(list_indices, coeff):
    """Bucket each expert's (u, v, coeff) entries by (v-chunk, u-chunk), pad
    buckets to PAD, and lay out partition-major (entry 128*g + p lands at
    column g of partition p's contiguous gather run)."""
    li = list_indices.astype(np.int64)
    uu = li // N
    vv = li % N
    u2 = np.zeros((E, EROW), np.float32)
    vm2 = np.full((E, EROW), -9.0, np.float32)
    cv2 = np.zeros((E, EROW), np.float32)
    for e in range(E):
        for j in range(NBLK):
            selj = vv[e] // 128 == j
            for ub in range(NBLK):
                sel = np.where(selj & (uu[e] // 128 == ub))[0]
                cnt = len(sel)
                assert cnt <= PAD, f"bucket overflow: e{e} j{j} ub{ub}: {cnt}"
                base = (NBLK * j + ub) * PAD
                u2[e, base : base + cnt] = uu[e, sel] - 128 * ub
                vm2[e, base : base + cnt] = vv[e, sel] - 128 * j
                cv2[e, base : base + cnt] = coeff[e, sel]
    # partition-major runs: table[e, p*COLS + g] = arr[e, 128*g + p]
    perm = np.array([128 * g + p for p in range(128) for g in range(COLS)])
    return u2[:, perm], vm2[:, perm], cv2[:, perm]


def _bf16(x):
    return np.ascontiguousarray(np.asarray(x, np.float32).astype(ml_dtypes.bfloat16))


def kernel(cls_token, W_router, b_router, coeff, list_indices):
    global LAST_RESULT
    cls_token = np.asarray(cls_token)
    W_router = np.asarray(W_router)
    b_router = np.asarray(b_router)
    coeff = np.asarray(coeff)
    list_indices = np.asarray(list_indices)
    assert cls_token.shape == (B, N) and coeff.shape == (E, NF)
    nc = _get_nc()
    ctpv, stpv, c384v = _host_tables()
    u2v, vm2v, cv2v = _host_entry_tables(list_indices, coeff)
    basesv = (np.arange(E, dtype=np.float32) * EROW).reshape(E, 1)
    wrr = np.ascontiguousarray(W_router, dtype=np.float32)
    brr = np.ascontiguousarray(b_router, dtype=np.float32)
    common = {
        "wr": wrr,
        "br": brr,
        "u2": u2v,
        "vm2": vm2v,
        "cv2": cv2v,
        "bases": basesv,
        "ctp": _bf16(ctpv),
        "stp": _bf16(stpv),
        "c384": _bf16(c384v),
    }
    in_maps = []
    for c in range(NCORES):
        m = dict(common)
        m["cls4"] = np.ascontiguousarray(
            cls_token[BPC * c : BPC * (c + 1)], dtype=np.float32
        )
        in_maps.append(m)
    res = run_bass_kernel_spmd(
        nc, in_maps, core_ids=list(range(NCORES)), trace=KERNEL_TRACE
    )
    LAST_RESULT = res
    out = np.concatenate([res.results[c]["out4"] for c in range(NCORES)], axis=0)
    return out
